# revision 31
# baseline (speedup 1.0000x reference)
"""Causal multi-head attention on 8 Trainium2 NeuronCores.

Sharding: data-parallel over batch (B=2) x tensor-parallel over heads
(16 heads -> 4 groups of 4). Core c handles batch c//4, head group c%4.
The host sums the 4 bf16 partial outputs per batch element in fp32.

All operand transposes (x^T, wq^T, wk^T, wv^T, wo^T) are done on the
HOST: the device never runs PE transposes, so phase 1 is pure
projection matmuls (dc-major, weights stationary across 4 s-chunks).

Matmuls run in bf16 (fp32 PSUM accumulation). The softmax row-sum is
fused into the o^T = [v|1s]^T P^T matmul via an appended ones column;
the reciprocal is taken on the single rowsum row and broadcast to 64
partitions with a K=1 bf16 matmul (fast: 2-byte moving operand).

Phase 2 processes q-chunks in order [0, 3, 2, 1]: the longest exp
chain (qc=3, 16 k-blocks) starts early while the shortest (qc=1)
forms the tail. Per (q-chunk, head-pair) only the S^T = k q^T matmuls
and the masked exp are emitted in the main loop; v-projection, AV
matmuls, normalization epilogues and output projections drain from a
FIFO work queue between them to keep the PE dense.
"""

import numpy as np
import ml_dtypes

import concourse.bacc as bacc
import concourse.bass as bass
import concourse.tile as tile
from concourse import bass_utils, mybir

B, S, D, H = 2, 2048, 1024, 16
DK = 64
NH = 4                 # heads per core
E = NH * DK            # 256: per-core head-dim slice
SCALE = 1.0 / 8.0      # 1/sqrt(DK)

F32 = mybir.dt.float32
BF16 = mybir.dt.bfloat16
FP8 = mybir.dt.float8e4
EXP_BIAS = -2.0        # exp(s/8 - 2): keeps P in fp8e4 range; cancels in
                       # the softmax normalization

QC = 512               # q-chunk (columns per attention tile)
NQC = S // QC          # 4
NKB = S // 128         # 16 k-blocks
QC_ORDER = [0, 3, 2, 1]


def _emit(tc, nc, xT_d, wqT_d, wkT_d, wvT_d, woT_d, yT_d, mask_d, ones_d):
    const = tc.alloc_tile_pool(name="const", bufs=1)
    perm = tc.alloc_tile_pool(name="perm", bufs=1)

    mask = const.tile([128, 128], BF16)
    nc.scalar.dma_start(out=mask, in_=mask_d)
    ones16 = const.tile([128, 64], BF16)
    nc.scalar.dma_start(out=ones16, in_=ones_d)

    # persistent sbuf tensors
    xT = perm.tile([128, 8, S], BF16)     # xT[p, dc, s] = x[s, dc*128+p]
    wqT = perm.tile([128, 8, E], BF16)    # wqT[p, dc, e] = wq[e, dc*128+p]
    wkT = perm.tile([128, 8, E], BF16)
    wvT = perm.tile([128, 8, E], BF16)
    woT = perm.tile([128, 2, D], BF16)    # woT[p, ec, o] = wo[o, ec*128+p]
    qT = perm.tile([128, 2, S], BF16)     # qT[p, hp, s]; p = hi*64+dk
    kT = perm.tile([128, 2, S], BF16)
    v_sb = perm.tile([128, NKB, NH, 128], BF16)  # [.., 64] = ones column

    # interleave weight + x chunk loads so the first projection pass can
    # start after the first (wq, wk, x-quarter) triple lands; x chunks are
    # split into s-quarters to match the dc-major consumption order
    for dc in range(8):
        nc.sync.dma_start(out=wqT[:, dc, :], in_=wqT_d[dc * 128:(dc + 1) * 128, :])
        nc.sync.dma_start(out=wkT[:, dc, :], in_=wkT_d[dc * 128:(dc + 1) * 128, :])
        nc.sync.dma_start(out=xT[:, dc, 0:S // 2],
                          in_=xT_d[dc * 128:(dc + 1) * 128, 0:S // 2])
        nc.scalar.dma_start(out=xT[:, dc, S // 2:S],
                            in_=xT_d[dc * 128:(dc + 1) * 128, S // 2:S])
    for dc in range(8):
        nc.scalar.dma_start(out=wvT[:, dc, :], in_=wvT_d[dc * 128:(dc + 1) * 128, :])
    for ec in range(2):
        nc.scalar.dma_start(out=woT[:, ec, :], in_=woT_d[ec * 128:(ec + 1) * 128, :])

    # ones column of v (written once; strided 3D AP)
    ones_ap = bass.AP(
        tensor=v_sb.tensor,
        offset=v_sb.offset + DK,
        ap=[v_sb.ap[0], [NH * 128, NKB], [128, NH]],
    )
    nc.vector.memset(ones_ap, 1.0)

    ncopy = [0]

    def copy(dst, src):
        # alternate psum->sbuf copies across DVE and ACT in phase 1
        if ncopy[0] % 2 == 0:
            nc.vector.tensor_copy(dst, src)
        else:
            nc.scalar.copy(dst, src)
        ncopy[0] += 1

    work = tc.alloc_tile_pool(name="work", bufs=3)
    small = tc.alloc_tile_pool(name="small", bufs=2)

    # ---- phase 1: q/k projections, dc-major (weights stationary
    # across the 4 s-chunks -> hardware may skip weight reloads) ----
    with tc.tile_pool(name="psP", bufs=1, space="PSUM") as psP:
        def proj_pass(w_t, outT, ec):
            ps = [psP.tile([128, QC], F32, tag="proj", bufs=8, name="psp")
                  for _ in range(4)]
            for dc in range(8):
                for sc in range(4):
                    nc.tensor.matmul(
                        ps[sc],
                        lhsT=w_t[:, dc, ec * 128:(ec + 1) * 128],
                        rhs=xT[:, dc, sc * QC:(sc + 1) * QC],
                        start=(dc == 0),
                        stop=(dc == 7),
                    )
            for sc in range(4):
                copy(outT[:, ec, sc * QC:(sc + 1) * QC], ps[sc])

        proj_pass(wqT, qT, 0)
        proj_pass(wkT, kT, 0)

    with tc.tile_pool(name="psS", bufs=1, space="PSUM") as ps_S, \
         tc.tile_pool(name="psO", bufs=1, space="PSUM") as ps_o, \
         tc.tile_pool(name="psY", bufs=1, space="PSUM") as ps_y:

        # ---- phase 2: attention + output projection, FIFO work queue ----
        workq = []  # (weight, closure); FIFO order guarantees v before AV

        def make_proj_unit(w_t, outT, ec, sc):
            def u():
                ps = ps_y.tile([128, QC], F32, tag="y", bufs=2, name="psu")
                for dc in range(8):
                    nc.tensor.matmul(
                        ps,
                        lhsT=w_t[:, dc, ec * 128:(ec + 1) * 128],
                        rhs=xT[:, dc, sc * QC:(sc + 1) * QC],
                        start=(dc == 0),
                        stop=(dc == 7),
                    )
                nc.vector.tensor_copy(outT[:, ec, sc * QC:(sc + 1) * QC], ps)
            return u

        def make_vproj(sblk):
            def u():
                ps = ps_y.tile([128, E], F32, tag="y", bufs=2, name="psv")
                for dc in range(8):
                    nc.tensor.matmul(
                        ps,
                        lhsT=xT[:, dc, sblk * 128:(sblk + 1) * 128],
                        rhs=wvT[:, dc, :],
                        start=(dc == 0),
                        stop=(dc == 7),
                    )
                # scatter 4 heads into [.., l, 0:64] (DVE: strided psum read)
                sap = bass.AP(
                    tensor=ps.tensor, offset=ps.offset,
                    ap=[ps.ap[0], [DK, NH], [1, DK]],
                )
                nc.vector.tensor_copy(v_sb[:, sblk, :, 0:DK], sap)
            return u

        for sblk in range(NKB):
            workq.append((6, make_vproj(sblk)))
        for w_t, outT in ((wqT, qT), (wkT, kT)):
            for sc in range(4):
                workq.append((10, make_proj_unit(w_t, outT, 1, sc)))

        # drain pacing: spread background work evenly over the QK slots
        TOTAL_SLOTS = 2 * sum(4 * (q + 1) for q in range(NQC))      # 80
        W_TOTAL = 6 * NKB + 10 * 8 + 2 * TOTAL_SLOTS + 2 * NQC + 3 * 8 * NQC
        slot_i = [0]
        drained = [0]
        qpop = [0]          # units popped (for watermark guard)
        nqueued = [len(workq)]

        def drain_one():
            w, u = workq.pop(0)
            u()
            drained[0] += w
            qpop[0] += 1
            return w

        def pump():
            slot_i[0] += 1
            target = (W_TOTAL * slot_i[0]) // TOTAL_SLOTS
            budget = min(14, target - drained[0])
            while workq and budget > 0:
                budget -= drain_one()

        def make_av(po_box, pts, kb, hp, kmax, qc):
            def av():
                if po_box[0] is None:
                    po_box[0] = (
                        ps_o.tile([128, QC], F32, tag="o", bufs=2, name="poA"),
                        ps_o.tile([128, QC], F32, tag="o", bufs=2, name="poB"),
                    )
                poA, poB = po_box[0]
                pT, cs = pts[kb]
                for hi, po in ((0, poA), (1, poB)):
                    # 128-col weights (v padded with 64 dead columns) keep
                    # FWL on; po rows 65:127 hold garbage and are never read
                    nc.tensor.matmul(
                        po[:, cs:512],
                        lhsT=v_sb[:, kb, 2 * hp + hi, :],
                        rhs=pT[:, hi, cs:512],
                        start=(kb == 0),
                        stop=(kb == kmax - 1),
                    )
            return av

        def make_epilogue(po_box, oT, hp, qc=None):
            def epi():
                poA, poB = po_box[0]
                # free the psum banks quickly with one copy per head
                oA_sb = small.tile([DK + 1, QC], F32, tag="osb", bufs=4)
                oB_sb = small.tile([DK + 1, QC], F32, tag="osb", bufs=4)
                nc.vector.tensor_copy(oA_sb, poA[0:DK + 1, :])
                nc.vector.tensor_copy(oB_sb, poB[0:DK + 1, :])
                for hi, o_sb in ((0, oA_sb), (1, oB_sb)):
                    # broadcast the rowsum row (partition 64) to 64
                    # partitions via K=1 bf16 matmul (2-byte moving operand
                    # streams 2x faster than fp32), then reciprocal
                    sum16 = small.tile([DK + 1, QC], BF16, tag="rec", bufs=2)
                    nc.vector.tensor_copy(sum16[DK:DK + 1, :], o_sb[DK:DK + 1, :])
                    ps_bc = ps_y.tile([64, QC], F32, tag="y", bufs=2, name="psbc")
                    nc.tensor.matmul(
                        ps_bc,
                        lhsT=ones16[64:65, :],
                        rhs=sum16[DK:DK + 1, :],
                        start=True,
                        stop=True,
                    )
                    rec = small.tile([64, QC], F32, tag="recf", bufs=2)
                    nc.vector.reciprocal_approx_fast(rec, ps_bc)
                    if hi == 0:
                        nc.vector.tensor_mul(oT[0:64, hp, :], o_sb[0:DK, :], rec)
                    else:
                        tmpB = small.tile([64, QC], BF16, tag="tmpB", bufs=2)
                        nc.vector.tensor_mul(tmpB, o_sb[0:DK, :], rec)
                        # partition shift 0-63 -> 64-127 via sbuf->sbuf DMA
                        nc.sync.dma_start(out=oT[64:128, hp, :], in_=tmpB)
            return epi

        def make_out_proj(qc, oT):
            units = []
            for dc in range(8):
                def u(dc=dc, qc=qc, oT=oT):
                    psy = ps_y.tile([128, QC], F32, tag="y", bufs=2, name="psy")
                    for ec in range(2):
                        nc.tensor.matmul(
                            psy,
                            lhsT=woT[:, ec, dc * 128:(dc + 1) * 128],
                            rhs=oT[:, ec, :],
                            start=(ec == 0),
                            stop=(ec == 1),
                        )
                    y_sb = work.tile([128, QC], BF16, tag="ysb", bufs=3)
                    if qc == 1:
                        nc.scalar.copy(y_sb, psy)
                    else:
                        nc.vector.tensor_copy(y_sb, psy)
                    nc.sync.dma_start(
                        out=yT_d[dc * 128:(dc + 1) * 128, qc * QC:(qc + 1) * QC],
                        in_=y_sb,
                    )
                units.append(u)
            return units

        marks = []  # cumulative queued-unit count at each chain's end
        CHAINS = [(0, 0), (3, 0), (0, 1), (3, 1), (2, 0), (2, 1), (1, 0), (1, 1)]
        N_PRE = len(workq)  # vproj + proj-ec1 units
        oTs = {}
        done_qc = set()
        for ci, (qc, hp) in enumerate(CHAINS):
            if qc not in oTs:
                oTs[qc] = work.tile([128, 2, QC], BF16, tag="oT", bufs=3, name="oT")
            oT = oTs[qc]
            kmax = 4 * (qc + 1)
            if True:
                # ec1 q/k projections must be emitted before any hp=1 chain
                if hp == 1:
                    while qpop[0] < N_PRE:
                        drain_one()
                # guard: chains up to N-2 fully drained before N emits, so
                # at most two chains' pT tiles (<=32) are live in the ring
                if len(marks) >= 2:
                    while qpop[0] < marks[-2]:
                        drain_one()
                pts = {}
                po_box = [None]
                for kb in range(kmax):
                    # S^T = k q^T, 2-head row-tiled pair, causally narrowed
                    cs = max(0, kb * 128 - qc * QC)
                    psS = ps_S.tile([128, 2, 512], F32, tag="S", bufs=2)
                    for hi in range(2):
                        nc.tensor.matmul(
                            psS[:, hi, cs:512],
                            lhsT=kT[hi * 64:(hi + 1) * 64, hp,
                                    kb * 128:(kb + 1) * 128],
                            rhs=qT[hi * 64:(hi + 1) * 64, hp,
                                   qc * QC + cs:(qc + 1) * QC],
                            start=True,
                            stop=True,
                        )
                    pT = work.tile([128, 2, 512], BF16, tag="pT", bufs=34)
                    pts[kb] = (pT, cs)
                    nc.scalar.activation(
                        pT[:, :, cs:512],
                        psS[:, :, cs:512],
                        mybir.ActivationFunctionType.Exp,
                        scale=SCALE,
                    )
                    if kb >= 4 * qc:  # diagonal band: zero the upper triangle
                        mask2 = bass.AP(
                            tensor=mask.tensor, offset=mask.offset,
                            ap=[mask.ap[0], [0, 2], mask.ap[1]],
                        )
                        nc.vector.tensor_mul(
                            pT[:, :, cs:cs + 128],
                            pT[:, :, cs:cs + 128],
                            mask2,
                        )
                    pump()
                # queue this phase's AV work + epilogue
                for kb in range(kmax):
                    workq.append((2, make_av(po_box, pts, kb, hp, kmax, qc)))
                workq.append((1, make_epilogue(po_box, oT, hp, qc=qc)))
                if qc in done_qc:
                    workq.extend((3, u) for u in make_out_proj(qc, oT))
                done_qc.add(qc)
                nqueued[0] = qpop[0] + len(workq)
                marks.append(nqueued[0])
        for _, u in workq:
            u()

    if _DBG:
        nc.sync.dma_start(out=_DBG["qdbg"], in_=qT[:, :, :])
        nc.sync.dma_start(out=_DBG["kdbg"], in_=kT[:, :, :])
        nc.sync.dma_start(out=_DBG["vdbg"], in_=v_sb[:, :, :, :])

    for p in [small, work, perm, const]:
        p.release()


_CACHE = {}
_DBG = {}


def _build():
    if "nc" in _CACHE:
        return _CACHE["nc"]
    nc = bacc.Bacc("TRN2", target_bir_lowering=False, debug=False, num_devices=8)
    import os
    if os.environ.get("KDBG"):
        _DBG["qdbg"] = nc.dram_tensor("qdbg", [128, 2, S], BF16, kind="ExternalOutput").ap()
        _DBG["kdbg"] = nc.dram_tensor("kdbg", [128, 2, S], BF16, kind="ExternalOutput").ap()
        _DBG["vdbg"] = nc.dram_tensor("vdbg", [128, NKB, NH, DK + 1], BF16, kind="ExternalOutput").ap()
        _DBG["edbg"] = nc.dram_tensor("edbg", [1, 3, S // 4], F32, kind="ExternalOutput").ap()
    xT_d = nc.dram_tensor("xT", [D, S], BF16, kind="ExternalInput").ap()
    wqT_d = nc.dram_tensor("wqT", [D, E], BF16, kind="ExternalInput").ap()
    wkT_d = nc.dram_tensor("wkT", [D, E], BF16, kind="ExternalInput").ap()
    wvT_d = nc.dram_tensor("wvT", [D, E], BF16, kind="ExternalInput").ap()
    woT_d = nc.dram_tensor("woT", [E, D], BF16, kind="ExternalInput").ap()
    yT_d = nc.dram_tensor("yT", [D, S], BF16, kind="ExternalOutput").ap()
    mask_d = nc.dram_tensor("maskc", [128, 128], BF16, kind="ExternalInput").ap()
    ones_d = nc.dram_tensor("onesc", [128, 64], BF16, kind="ExternalInput").ap()
    with tile.TileContext(nc) as tc:
        _emit(tc, nc, xT_d, wqT_d, wkT_d, wvT_d, woT_d, yT_d, mask_d, ones_d)
    nc.compile()
    _CACHE["nc"] = nc
    return nc


_r = np.arange(128)
_MASK = np.where(_r[:, None] <= _r[None, :], 1.0, 0.0).astype(ml_dtypes.bfloat16)
_ONES16 = np.ones((128, 64), dtype=ml_dtypes.bfloat16)

LAST_RESULT = None


def kernel(x, wq, wk, wv, wo):
    global LAST_RESULT
    nc = _build()
    bf = ml_dtypes.bfloat16
    x16 = np.asarray(x, dtype=np.float32).astype(bf)
    wq16 = np.asarray(wq, dtype=np.float32).astype(bf)
    wk16 = np.asarray(wk, dtype=np.float32).astype(bf)
    wv16 = np.asarray(wv, dtype=np.float32).astype(bf)
    wo16 = np.asarray(wo, dtype=np.float32).astype(bf)

    xTb = [np.ascontiguousarray(x16[b].T) for b in range(B)]
    in_maps = []
    for c in range(8):
        b, g = c // 4, c % 4
        rows = slice(g * E, (g + 1) * E)
        in_maps.append({
            "xT": xTb[b],
            "wqT": np.ascontiguousarray(wq16[rows].T),
            "wkT": np.ascontiguousarray(wk16[rows].T),
            "wvT": np.ascontiguousarray(wv16[rows].T),
            "woT": np.ascontiguousarray(wo16[:, rows].T),
            "maskc": _MASK,
            "onesc": _ONES16,
        })

    res = bass_utils.run_bass_kernel_spmd(nc, in_maps, core_ids=list(range(8)))
    LAST_RESULT = res

    y = np.empty((B, S, D), dtype=np.float32)
    for b in range(B):
        acc = res.results[4 * b]["yT"].astype(np.float32)
        for g in range(1, 4):
            acc += res.results[4 * b + g]["yT"].astype(np.float32)
        y[b] = acc.T
    return y


# revision 32
# speedup vs baseline: 1.0462x; 1.0462x over previous
"""Causal multi-head attention on 8 Trainium2 NeuronCores.

Sharding: data-parallel over batch (B=2) x tensor-parallel over heads
(16 heads -> 4 groups of 4). Core c handles batch c//4, head group c%4.
The host sums the 4 bf16 partial outputs per batch element in fp32.

All operand transposes (x^T, wq^T, wk^T, wv^T, wo^T) are done on the
HOST: the device never runs PE transposes, so phase 1 is pure
projection matmuls (dc-major, weights stationary across 4 s-chunks).

Matmuls run in bf16 (fp32 PSUM accumulation). The softmax row-sum is
fused into the o^T = [v|1s]^T P^T matmul via an appended ones column;
the reciprocal is taken on the single rowsum row and broadcast to 64
partitions with a K=1 bf16 matmul (fast: 2-byte moving operand).

Phase 2 processes q-chunks in order [0, 3, 2, 1]: the longest exp
chain (qc=3, 16 k-blocks) starts early while the shortest (qc=1)
forms the tail. Per (q-chunk, head-pair) only the S^T = k q^T matmuls
and the masked exp are emitted in the main loop; v-projection, AV
matmuls, normalization epilogues and output projections drain from a
FIFO work queue between them to keep the PE dense.
"""

import numpy as np
import ml_dtypes

import concourse.bacc as bacc
import concourse.bass as bass
import concourse.tile as tile
from concourse import bass_utils, mybir

B, S, D, H = 2, 2048, 1024, 16
DK = 64
NH = 4                 # heads per core
E = NH * DK            # 256: per-core head-dim slice
SCALE = 1.0 / 8.0      # 1/sqrt(DK)

F32 = mybir.dt.float32
BF16 = mybir.dt.bfloat16
FP8 = mybir.dt.float8e4
EXP_BIAS = -2.0        # exp(s/8 - 2): keeps P in fp8e4 range; cancels in
                       # the softmax normalization

QC = 512               # q-chunk (columns per attention tile)
NQC = S // QC          # 4
NKB = S // 128         # 16 k-blocks
QC_ORDER = [0, 3, 2, 1]


def _emit(tc, nc, xT_d, wqT_d, wkT_d, wvT_d, woT_d, yT_d, mask_d, ones_d):
    const = tc.alloc_tile_pool(name="const", bufs=1)
    perm = tc.alloc_tile_pool(name="perm", bufs=1)

    mask = const.tile([128, 128], BF16)
    nc.scalar.dma_start(out=mask, in_=mask_d)
    ones16 = const.tile([128, 64], BF16)
    nc.scalar.dma_start(out=ones16, in_=ones_d)

    # persistent sbuf tensors
    xT = perm.tile([128, 8, S], BF16)     # xT[p, dc, s] = x[s, dc*128+p]
    wqT = perm.tile([128, 8, E], BF16)    # wqT[p, dc, e] = wq[e, dc*128+p]
    wkT = perm.tile([128, 8, E], BF16)
    wvT = perm.tile([128, 8, E], BF16)
    woT = perm.tile([128, 2, D], BF16)    # woT[p, ec, o] = wo[o, ec*128+p]
    qT = perm.tile([128, 2, S], BF16)     # qT[p, hp, s]; p = hi*64+dk
    kT = perm.tile([128, 2, S], BF16)
    v_sb = perm.tile([128, NKB, NH, 128], BF16)  # [.., 64] = ones column

    # interleave weight + x chunk loads so the first projection pass can
    # start after the first (wq, wk, x-quarter) triple lands; x chunks are
    # split into s-quarters to match the dc-major consumption order
    for dc in range(8):
        nc.sync.dma_start(out=wqT[:, dc, :], in_=wqT_d[dc * 128:(dc + 1) * 128, :])
        nc.sync.dma_start(out=wkT[:, dc, :], in_=wkT_d[dc * 128:(dc + 1) * 128, :])
        nc.sync.dma_start(out=xT[:, dc, :], in_=xT_d[dc * 128:(dc + 1) * 128, :])
    for dc in range(8):
        nc.sync.dma_start(out=wvT[:, dc, :], in_=wvT_d[dc * 128:(dc + 1) * 128, :])
    for ec in range(2):
        nc.sync.dma_start(out=woT[:, ec, :], in_=woT_d[ec * 128:(ec + 1) * 128, :])

    # ones column of v (written once; strided 3D AP)
    ones_ap = bass.AP(
        tensor=v_sb.tensor,
        offset=v_sb.offset + DK,
        ap=[v_sb.ap[0], [NH * 128, NKB], [128, NH]],
    )
    nc.vector.memset(ones_ap, 1.0)

    ncopy = [0]

    def copy(dst, src):
        # alternate psum->sbuf copies across DVE and ACT in phase 1
        if ncopy[0] % 2 == 0:
            nc.vector.tensor_copy(dst, src)
        else:
            nc.scalar.copy(dst, src)
        ncopy[0] += 1

    work = tc.alloc_tile_pool(name="work", bufs=3)
    small = tc.alloc_tile_pool(name="small", bufs=2)

    # ---- phase 1: q/k projections, dc-major (weights stationary
    # across the 4 s-chunks -> hardware may skip weight reloads) ----
    with tc.tile_pool(name="psP", bufs=1, space="PSUM") as psP:
        def proj_pass(w_t, outT, ec):
            ps = [psP.tile([128, QC], F32, tag="proj", bufs=8, name="psp")
                  for _ in range(4)]
            for dc in range(8):
                for sc in range(4):
                    nc.tensor.matmul(
                        ps[sc],
                        lhsT=w_t[:, dc, ec * 128:(ec + 1) * 128],
                        rhs=xT[:, dc, sc * QC:(sc + 1) * QC],
                        start=(dc == 0),
                        stop=(dc == 7),
                    )
            for sc in range(4):
                copy(outT[:, ec, sc * QC:(sc + 1) * QC], ps[sc])

        proj_pass(wqT, qT, 0)
        proj_pass(wkT, kT, 0)

    with tc.tile_pool(name="psS", bufs=1, space="PSUM") as ps_S, \
         tc.tile_pool(name="psO", bufs=1, space="PSUM") as ps_o, \
         tc.tile_pool(name="psY", bufs=1, space="PSUM") as ps_y:

        # ---- phase 2: attention + output projection, FIFO work queue ----
        workq = []  # (weight, closure); FIFO order guarantees v before AV

        def make_proj_unit(w_t, outT, ec, sc):
            def u():
                ps = ps_y.tile([128, QC], F32, tag="y", bufs=2, name="psu")
                for dc in range(8):
                    nc.tensor.matmul(
                        ps,
                        lhsT=w_t[:, dc, ec * 128:(ec + 1) * 128],
                        rhs=xT[:, dc, sc * QC:(sc + 1) * QC],
                        start=(dc == 0),
                        stop=(dc == 7),
                    )
                nc.vector.tensor_copy(outT[:, ec, sc * QC:(sc + 1) * QC], ps)
            return u

        def make_vproj(sblk):
            def u():
                ps = ps_y.tile([128, E], F32, tag="y", bufs=2, name="psv")
                for dc in range(8):
                    nc.tensor.matmul(
                        ps,
                        lhsT=xT[:, dc, sblk * 128:(sblk + 1) * 128],
                        rhs=wvT[:, dc, :],
                        start=(dc == 0),
                        stop=(dc == 7),
                    )
                # scatter 4 heads into [.., l, 0:64] (DVE: strided psum read)
                sap = bass.AP(
                    tensor=ps.tensor, offset=ps.offset,
                    ap=[ps.ap[0], [DK, NH], [1, DK]],
                )
                nc.vector.tensor_copy(v_sb[:, sblk, :, 0:DK], sap)
            return u

        for sblk in range(NKB):
            workq.append((6, make_vproj(sblk)))
        for w_t, outT in ((wqT, qT), (wkT, kT)):
            for sc in range(4):
                workq.append((10, make_proj_unit(w_t, outT, 1, sc)))

        # drain pacing: spread background work evenly over the QK slots
        TOTAL_SLOTS = 2 * sum(4 * (q + 1) for q in range(NQC))      # 80
        W_TOTAL = 6 * NKB + 10 * 8 + 2 * TOTAL_SLOTS + 2 * NQC + 3 * 8 * NQC
        slot_i = [0]
        drained = [0]
        qpop = [0]          # units popped (for watermark guard)
        nqueued = [len(workq)]

        def drain_one():
            w, u = workq.pop(0)
            u()
            drained[0] += w
            qpop[0] += 1
            return w

        def pump():
            slot_i[0] += 1
            target = (W_TOTAL * slot_i[0]) // TOTAL_SLOTS
            budget = min(14, target - drained[0])
            while workq and budget > 0:
                budget -= drain_one()

        def make_av(po_box, pts, kb, hp, kmax, qc):
            def av():
                if po_box[0] is None:
                    po_box[0] = (
                        ps_o.tile([128, QC], F32, tag="o", bufs=2, name="poA"),
                        ps_o.tile([128, QC], F32, tag="o", bufs=2, name="poB"),
                    )
                poA, poB = po_box[0]
                pT, cs = pts[kb]
                for hi, po in ((0, poA), (1, poB)):
                    # 128-col weights (v padded with 64 dead columns) keep
                    # FWL on; po rows 65:127 hold garbage and are never read
                    nc.tensor.matmul(
                        po[:, cs:512],
                        lhsT=v_sb[:, kb, 2 * hp + hi, :],
                        rhs=pT[:, hi, cs:512],
                        start=(kb == 0),
                        stop=(kb == kmax - 1),
                    )
            return av

        def make_epilogue(po_box, oT, hp, qc=None):
            def epi():
                poA, poB = po_box[0]
                # free the psum banks quickly with one copy per head
                oA_sb = small.tile([DK + 1, QC], F32, tag="osb", bufs=4)
                oB_sb = small.tile([DK + 1, QC], F32, tag="osb", bufs=4)
                nc.vector.tensor_copy(oA_sb, poA[0:DK + 1, :])
                nc.vector.tensor_copy(oB_sb, poB[0:DK + 1, :])
                for hi, o_sb in ((0, oA_sb), (1, oB_sb)):
                    # broadcast the rowsum row (partition 64) to 64
                    # partitions via K=1 bf16 matmul (2-byte moving operand
                    # streams 2x faster than fp32), then reciprocal
                    sum16 = small.tile([DK + 1, QC], BF16, tag="rec", bufs=2)
                    nc.vector.tensor_copy(sum16[DK:DK + 1, :], o_sb[DK:DK + 1, :])
                    ps_bc = ps_y.tile([64, QC], F32, tag="y", bufs=2, name="psbc")
                    nc.tensor.matmul(
                        ps_bc,
                        lhsT=ones16[64:65, :],
                        rhs=sum16[DK:DK + 1, :],
                        start=True,
                        stop=True,
                    )
                    rec = small.tile([64, QC], F32, tag="recf", bufs=2)
                    nc.vector.reciprocal_approx_fast(rec, ps_bc)
                    if hi == 0:
                        nc.vector.tensor_mul(oT[0:64, hp, :], o_sb[0:DK, :], rec)
                    else:
                        tmpB = small.tile([64, QC], BF16, tag="tmpB", bufs=2)
                        nc.vector.tensor_mul(tmpB, o_sb[0:DK, :], rec)
                        # partition shift 0-63 -> 64-127 via sbuf->sbuf DMA
                        nc.sync.dma_start(out=oT[64:128, hp, :], in_=tmpB)
            return epi

        def make_out_proj(qc, oT):
            units = []
            for dc in range(8):
                def u(dc=dc, qc=qc, oT=oT):
                    psy = ps_y.tile([128, QC], F32, tag="y", bufs=2, name="psy")
                    for ec in range(2):
                        nc.tensor.matmul(
                            psy,
                            lhsT=woT[:, ec, dc * 128:(dc + 1) * 128],
                            rhs=oT[:, ec, :],
                            start=(ec == 0),
                            stop=(ec == 1),
                        )
                    y_sb = work.tile([128, QC], BF16, tag="ysb", bufs=3)
                    nc.vector.tensor_copy(y_sb, psy)
                    nc.sync.dma_start(
                        out=yT_d[dc * 128:(dc + 1) * 128, qc * QC:(qc + 1) * QC],
                        in_=y_sb,
                    )
                units.append(u)
            return units

        marks = []  # cumulative queued-unit count at each chain's end
        CHAINS = [(0, 0), (3, 0), (0, 1), (3, 1), (2, 0), (2, 1), (1, 0), (1, 1)]
        N_PRE = len(workq)  # vproj + proj-ec1 units
        oTs = {}
        done_qc = set()
        for ci, (qc, hp) in enumerate(CHAINS):
            if qc not in oTs:
                oTs[qc] = work.tile([128, 2, QC], BF16, tag="oT", bufs=3, name="oT")
            oT = oTs[qc]
            kmax = 4 * (qc + 1)
            if True:
                # ec1 q/k projections must be emitted before any hp=1 chain
                if hp == 1:
                    while qpop[0] < N_PRE:
                        drain_one()
                # guard: chains up to N-2 fully drained before N emits, so
                # at most two chains' pT tiles (<=32) are live in the ring
                if len(marks) >= 2:
                    while qpop[0] < marks[-2]:
                        drain_one()
                pts = {}
                po_box = [None]
                for kb in range(kmax):
                    # S^T = k q^T, 2-head row-tiled pair, causally narrowed
                    cs = max(0, kb * 128 - qc * QC)
                    psS = ps_S.tile([128, 2, 512], F32, tag="S", bufs=2)
                    for hi in range(2):
                        nc.tensor.matmul(
                            psS[:, hi, cs:512],
                            lhsT=kT[hi * 64:(hi + 1) * 64, hp,
                                    kb * 128:(kb + 1) * 128],
                            rhs=qT[hi * 64:(hi + 1) * 64, hp,
                                   qc * QC + cs:(qc + 1) * QC],
                            start=True,
                            stop=True,
                        )
                    pT = work.tile([128, 2, 512], BF16, tag="pT", bufs=34)
                    pts[kb] = (pT, cs)
                    nc.scalar.activation(
                        pT[:, :, cs:512],
                        psS[:, :, cs:512],
                        mybir.ActivationFunctionType.Exp,
                        scale=SCALE,
                    )
                    if kb >= 4 * qc:  # diagonal band: zero the upper triangle
                        mask2 = bass.AP(
                            tensor=mask.tensor, offset=mask.offset,
                            ap=[mask.ap[0], [0, 2], mask.ap[1]],
                        )
                        nc.vector.tensor_mul(
                            pT[:, :, cs:cs + 128],
                            pT[:, :, cs:cs + 128],
                            mask2,
                        )
                    pump()
                # queue this phase's AV work + epilogue
                for kb in range(kmax):
                    workq.append((2, make_av(po_box, pts, kb, hp, kmax, qc)))
                workq.append((1, make_epilogue(po_box, oT, hp, qc=qc)))
                if qc in done_qc:
                    workq.extend((3, u) for u in make_out_proj(qc, oT))
                done_qc.add(qc)
                nqueued[0] = qpop[0] + len(workq)
                marks.append(nqueued[0])
        for _, u in workq:
            u()

    if _DBG:
        nc.sync.dma_start(out=_DBG["qdbg"], in_=qT[:, :, :])
        nc.sync.dma_start(out=_DBG["kdbg"], in_=kT[:, :, :])
        nc.sync.dma_start(out=_DBG["vdbg"], in_=v_sb[:, :, :, :])

    for p in [small, work, perm, const]:
        p.release()


_CACHE = {}
_DBG = {}


def _build():
    if "nc" in _CACHE:
        return _CACHE["nc"]
    nc = bacc.Bacc("TRN2", target_bir_lowering=False, debug=False, num_devices=8)
    import os
    if os.environ.get("KDBG"):
        _DBG["qdbg"] = nc.dram_tensor("qdbg", [128, 2, S], BF16, kind="ExternalOutput").ap()
        _DBG["kdbg"] = nc.dram_tensor("kdbg", [128, 2, S], BF16, kind="ExternalOutput").ap()
        _DBG["vdbg"] = nc.dram_tensor("vdbg", [128, NKB, NH, DK + 1], BF16, kind="ExternalOutput").ap()
        _DBG["edbg"] = nc.dram_tensor("edbg", [1, 3, S // 4], F32, kind="ExternalOutput").ap()
    xT_d = nc.dram_tensor("xT", [D, S], BF16, kind="ExternalInput").ap()
    wqT_d = nc.dram_tensor("wqT", [D, E], BF16, kind="ExternalInput").ap()
    wkT_d = nc.dram_tensor("wkT", [D, E], BF16, kind="ExternalInput").ap()
    wvT_d = nc.dram_tensor("wvT", [D, E], BF16, kind="ExternalInput").ap()
    woT_d = nc.dram_tensor("woT", [E, D], BF16, kind="ExternalInput").ap()
    yT_d = nc.dram_tensor("yT", [D, S], BF16, kind="ExternalOutput").ap()
    mask_d = nc.dram_tensor("maskc", [128, 128], BF16, kind="ExternalInput").ap()
    ones_d = nc.dram_tensor("onesc", [128, 64], BF16, kind="ExternalInput").ap()
    with tile.TileContext(nc) as tc:
        _emit(tc, nc, xT_d, wqT_d, wkT_d, wvT_d, woT_d, yT_d, mask_d, ones_d)
    nc.compile()
    _CACHE["nc"] = nc
    return nc


_r = np.arange(128)
_MASK = np.where(_r[:, None] <= _r[None, :], 1.0, 0.0).astype(ml_dtypes.bfloat16)
_ONES16 = np.ones((128, 64), dtype=ml_dtypes.bfloat16)

LAST_RESULT = None


def kernel(x, wq, wk, wv, wo):
    global LAST_RESULT
    nc = _build()
    bf = ml_dtypes.bfloat16
    x16 = np.asarray(x, dtype=np.float32).astype(bf)
    wq16 = np.asarray(wq, dtype=np.float32).astype(bf)
    wk16 = np.asarray(wk, dtype=np.float32).astype(bf)
    wv16 = np.asarray(wv, dtype=np.float32).astype(bf)
    wo16 = np.asarray(wo, dtype=np.float32).astype(bf)

    xTb = [np.ascontiguousarray(x16[b].T) for b in range(B)]
    in_maps = []
    for c in range(8):
        b, g = c // 4, c % 4
        rows = slice(g * E, (g + 1) * E)
        in_maps.append({
            "xT": xTb[b],
            "wqT": np.ascontiguousarray(wq16[rows].T),
            "wkT": np.ascontiguousarray(wk16[rows].T),
            "wvT": np.ascontiguousarray(wv16[rows].T),
            "woT": np.ascontiguousarray(wo16[:, rows].T),
            "maskc": _MASK,
            "onesc": _ONES16,
        })

    res = bass_utils.run_bass_kernel_spmd(nc, in_maps, core_ids=list(range(8)))
    LAST_RESULT = res

    y = np.empty((B, S, D), dtype=np.float32)
    for b in range(B):
        acc = res.results[4 * b]["yT"].astype(np.float32)
        for g in range(1, 4):
            acc += res.results[4 * b + g]["yT"].astype(np.float32)
        y[b] = acc.T
    return y


# revision 33
# speedup vs baseline: 1.0562x; 1.0096x over previous
"""Causal multi-head attention on 8 Trainium2 NeuronCores.

Sharding: data-parallel over batch (B=2) x tensor-parallel over heads
(16 heads -> 4 groups of 4). Core c handles batch c//4, head group c%4.
The host sums the 4 bf16 partial outputs per batch element in fp32.

All operand transposes (x^T, wq^T, wk^T, wv^T, wo^T) are done on the
HOST: the device never runs PE transposes, so phase 1 is pure
projection matmuls (dc-major, weights stationary across 4 s-chunks).

Matmuls run in bf16 (fp32 PSUM accumulation). The softmax row-sum is
fused into the o^T = [v|1s]^T P^T matmul via an appended ones column;
the reciprocal is taken on the single rowsum row and broadcast to 64
partitions with a K=1 bf16 matmul (fast: 2-byte moving operand).

Phase 2 processes q-chunks in order [0, 3, 2, 1]: the longest exp
chain (qc=3, 16 k-blocks) starts early while the shortest (qc=1)
forms the tail. Per (q-chunk, head-pair) only the S^T = k q^T matmuls
and the masked exp are emitted in the main loop; v-projection, AV
matmuls, normalization epilogues and output projections drain from a
FIFO work queue between them to keep the PE dense.
"""

import numpy as np
import ml_dtypes

import concourse.bacc as bacc
import concourse.bass as bass
import concourse.tile as tile
from concourse import bass_utils, mybir

B, S, D, H = 2, 2048, 1024, 16
DK = 64
NH = 4                 # heads per core
E = NH * DK            # 256: per-core head-dim slice
SCALE = 1.0 / 8.0      # 1/sqrt(DK)

F32 = mybir.dt.float32
BF16 = mybir.dt.bfloat16
FP8 = mybir.dt.float8e4
EXP_BIAS = -2.0        # exp(s/8 - 2): keeps P in fp8e4 range; cancels in
                       # the softmax normalization

QC = 512               # q-chunk (columns per attention tile)
NQC = S // QC          # 4
NKB = S // 128         # 16 k-blocks
QC_ORDER = [0, 3, 2, 1]


def _emit(tc, nc, xT_d, wqT_d, wkT_d, wvT_d, woT_d, yT_d, mask_d, ones_d):
    const = tc.alloc_tile_pool(name="const", bufs=1)
    perm = tc.alloc_tile_pool(name="perm", bufs=1)

    mask = const.tile([128, 128], BF16)
    nc.scalar.dma_start(out=mask, in_=mask_d)
    ones16 = const.tile([128, 64], BF16)
    nc.scalar.dma_start(out=ones16, in_=ones_d)

    # persistent sbuf tensors
    xT = perm.tile([128, 8, S], BF16)     # xT[p, dc, s] = x[s, dc*128+p]
    wqT = perm.tile([128, 8, E], BF16)    # wqT[p, dc, e] = wq[e, dc*128+p]
    wkT = perm.tile([128, 8, E], BF16)
    wvT = perm.tile([128, 8, E], BF16)
    woT = perm.tile([128, 2, D], BF16)    # woT[p, ec, o] = wo[o, ec*128+p]
    qT = perm.tile([128, 2, S], BF16)     # qT[p, hp, s]; p = hi*64+dk
    kT = perm.tile([128, 2, S], BF16)
    v_sb = perm.tile([128, NKB, NH, 128], BF16)  # [.., 64] = ones column

    # interleave weight + x chunk loads so the first projection pass can
    # start after the first (wq, wk, x-quarter) triple lands; x chunks are
    # split into s-quarters to match the dc-major consumption order
    for dc in range(8):
        nc.sync.dma_start(out=wqT[:, dc, :], in_=wqT_d[dc * 128:(dc + 1) * 128, :])
        nc.sync.dma_start(out=wkT[:, dc, :], in_=wkT_d[dc * 128:(dc + 1) * 128, :])
        nc.sync.dma_start(out=xT[:, dc, :], in_=xT_d[dc * 128:(dc + 1) * 128, :])
    for dc in range(8):
        nc.sync.dma_start(out=wvT[:, dc, :], in_=wvT_d[dc * 128:(dc + 1) * 128, :])
    for ec in range(2):
        nc.sync.dma_start(out=woT[:, ec, :], in_=woT_d[ec * 128:(ec + 1) * 128, :])

    # ones column of v (written once; strided 3D AP)
    ones_ap = bass.AP(
        tensor=v_sb.tensor,
        offset=v_sb.offset + DK,
        ap=[v_sb.ap[0], [NH * 128, NKB], [128, NH]],
    )
    nc.vector.memset(ones_ap, 1.0)

    ncopy = [0]

    def copy(dst, src):
        # alternate psum->sbuf copies across DVE and ACT in phase 1
        if ncopy[0] % 2 == 0:
            nc.vector.tensor_copy(dst, src)
        else:
            nc.scalar.copy(dst, src)
        ncopy[0] += 1

    work = tc.alloc_tile_pool(name="work", bufs=3)
    small = tc.alloc_tile_pool(name="small", bufs=2)

    # ---- phase 1: q/k projections, dc-major (weights stationary
    # across the 4 s-chunks -> hardware may skip weight reloads) ----
    with tc.tile_pool(name="psP", bufs=1, space="PSUM") as psP:
        def proj_pass(w_t, outT, ec):
            ps = [psP.tile([128, QC], F32, tag="proj", bufs=8, name="psp")
                  for _ in range(4)]
            for dc in range(8):
                for sc in range(4):
                    nc.tensor.matmul(
                        ps[sc],
                        lhsT=w_t[:, dc, ec * 128:(ec + 1) * 128],
                        rhs=xT[:, dc, sc * QC:(sc + 1) * QC],
                        start=(dc == 0),
                        stop=(dc == 7),
                    )
            for sc in range(4):
                copy(outT[:, ec, sc * QC:(sc + 1) * QC], ps[sc])

        proj_pass(wqT, qT, 0)
        proj_pass(wkT, kT, 0)

    with tc.tile_pool(name="psS", bufs=1, space="PSUM") as ps_S, \
         tc.tile_pool(name="psO", bufs=1, space="PSUM") as ps_o, \
         tc.tile_pool(name="psY", bufs=1, space="PSUM") as ps_y:

        # ---- phase 2: attention + output projection, FIFO work queue ----
        workq = []  # (weight, closure); FIFO order guarantees v before AV

        def make_proj_unit(w_t, outT, ec, sc):
            def u():
                ps = ps_y.tile([128, QC], F32, tag="y", bufs=2, name="psu")
                for dc in range(8):
                    nc.tensor.matmul(
                        ps,
                        lhsT=w_t[:, dc, ec * 128:(ec + 1) * 128],
                        rhs=xT[:, dc, sc * QC:(sc + 1) * QC],
                        start=(dc == 0),
                        stop=(dc == 7),
                    )
                nc.vector.tensor_copy(outT[:, ec, sc * QC:(sc + 1) * QC], ps)
            return u

        def make_vproj(sblk):
            def u():
                ps = ps_y.tile([128, E], F32, tag="y", bufs=2, name="psv")
                for dc in range(8):
                    nc.tensor.matmul(
                        ps,
                        lhsT=xT[:, dc, sblk * 128:(sblk + 1) * 128],
                        rhs=wvT[:, dc, :],
                        start=(dc == 0),
                        stop=(dc == 7),
                    )
                # scatter 4 heads into [.., l, 0:64] (DVE: strided psum read)
                sap = bass.AP(
                    tensor=ps.tensor, offset=ps.offset,
                    ap=[ps.ap[0], [DK, NH], [1, DK]],
                )
                nc.vector.tensor_copy(v_sb[:, sblk, :, 0:DK], sap)
            return u

        for sblk in range(NKB):
            workq.append((6, make_vproj(sblk)))
        for w_t, outT in ((wqT, qT), (wkT, kT)):
            for sc in range(4):
                workq.append((10, make_proj_unit(w_t, outT, 1, sc)))

        # drain pacing: spread background work evenly over the QK slots
        TOTAL_SLOTS = 2 * sum(4 * (q + 1) for q in range(NQC))      # 80
        W_TOTAL = 6 * NKB + 10 * 8 + 2 * TOTAL_SLOTS + 2 * NQC + 3 * 8 * NQC
        slot_i = [0]
        drained = [0]
        qpop = [0]          # units popped (for watermark guard)
        nqueued = [len(workq)]

        def drain_one():
            w, u = workq.pop(0)
            u()
            drained[0] += w
            qpop[0] += 1
            return w

        def pump():
            slot_i[0] += 1
            target = (W_TOTAL * slot_i[0]) // TOTAL_SLOTS
            budget = min(14, target - drained[0])
            while workq and budget > 0:
                budget -= drain_one()

        def make_av(po_box, pts, kb, hp, kmax, qc):
            def av():
                if po_box[0] is None:
                    po_box[0] = (
                        ps_o.tile([128, QC], F32, tag="o", bufs=2, name="poA"),
                        ps_o.tile([128, QC], F32, tag="o", bufs=2, name="poB"),
                    )
                poA, poB = po_box[0]
                pT, cs = pts[kb]
                for hi, po in ((0, poA), (1, poB)):
                    # 128-col weights (v padded with 64 dead columns) keep
                    # FWL on; po rows 65:127 hold garbage and are never read
                    nc.tensor.matmul(
                        po[:, cs:512],
                        lhsT=v_sb[:, kb, 2 * hp + hi, :],
                        rhs=pT[:, hi, cs:512],
                        start=(kb == 0),
                        stop=(kb == kmax - 1),
                    )
            return av

        def make_epilogue(po_box, oT, hp, qc=None):
            def epi():
                poA, poB = po_box[0]
                # free the psum banks quickly with one copy per head
                oA_sb = small.tile([DK + 1, QC], F32, tag="osb", bufs=4)
                oB_sb = small.tile([DK + 1, QC], F32, tag="osb", bufs=4)
                if qc == 1:
                    # tail chain: ACT's exp queue is empty by now
                    nc.scalar.copy(oA_sb, poA[0:DK + 1, :])
                    nc.scalar.copy(oB_sb, poB[0:DK + 1, :])
                else:
                    nc.vector.tensor_copy(oA_sb, poA[0:DK + 1, :])
                    nc.vector.tensor_copy(oB_sb, poB[0:DK + 1, :])
                for hi, o_sb in ((0, oA_sb), (1, oB_sb)):
                    # broadcast the rowsum row (partition 64) to 64
                    # partitions via K=1 bf16 matmul (2-byte moving operand
                    # streams 2x faster than fp32), then reciprocal
                    sum16 = small.tile([DK + 1, QC], BF16, tag="rec", bufs=2)
                    nc.vector.tensor_copy(sum16[DK:DK + 1, :], o_sb[DK:DK + 1, :])
                    ps_bc = ps_y.tile([64, QC], F32, tag="y", bufs=2, name="psbc")
                    nc.tensor.matmul(
                        ps_bc,
                        lhsT=ones16[64:65, :],
                        rhs=sum16[DK:DK + 1, :],
                        start=True,
                        stop=True,
                    )
                    rec = small.tile([64, QC], F32, tag="recf", bufs=2)
                    nc.vector.reciprocal_approx_fast(rec, ps_bc)
                    if hi == 0:
                        nc.vector.tensor_mul(oT[0:64, hp, :], o_sb[0:DK, :], rec)
                    else:
                        tmpB = small.tile([64, QC], BF16, tag="tmpB", bufs=2)
                        nc.vector.tensor_mul(tmpB, o_sb[0:DK, :], rec)
                        # partition shift 0-63 -> 64-127 via sbuf->sbuf DMA
                        nc.sync.dma_start(out=oT[64:128, hp, :], in_=tmpB)
            return epi

        def make_out_proj(qc, oT):
            units = []
            for dc in range(8):
                def u(dc=dc, qc=qc, oT=oT):
                    psy = ps_y.tile([128, QC], F32, tag="y", bufs=2, name="psy")
                    for ec in range(2):
                        nc.tensor.matmul(
                            psy,
                            lhsT=woT[:, ec, dc * 128:(dc + 1) * 128],
                            rhs=oT[:, ec, :],
                            start=(ec == 0),
                            stop=(ec == 1),
                        )
                    y_sb = work.tile([128, QC], BF16, tag="ysb", bufs=3)
                    if qc == 1:
                        nc.scalar.copy(y_sb, psy)
                    else:
                        nc.vector.tensor_copy(y_sb, psy)
                    nc.sync.dma_start(
                        out=yT_d[dc * 128:(dc + 1) * 128, qc * QC:(qc + 1) * QC],
                        in_=y_sb,
                    )
                units.append(u)
            return units

        marks = []  # cumulative queued-unit count at each chain's end
        CHAINS = [(0, 0), (3, 0), (0, 1), (3, 1), (2, 0), (2, 1), (1, 0), (1, 1)]
        N_PRE = len(workq)  # vproj + proj-ec1 units
        oTs = {}
        done_qc = set()
        for ci, (qc, hp) in enumerate(CHAINS):
            if qc not in oTs:
                oTs[qc] = work.tile([128, 2, QC], BF16, tag="oT", bufs=3, name="oT")
            oT = oTs[qc]
            kmax = 4 * (qc + 1)
            if True:
                # ec1 q/k projections must be emitted before any hp=1 chain
                if hp == 1:
                    while qpop[0] < N_PRE:
                        drain_one()
                # guard: chains up to N-2 fully drained before N emits, so
                # at most two chains' pT tiles (<=32) are live in the ring
                if len(marks) >= 2:
                    while qpop[0] < marks[-2]:
                        drain_one()
                pts = {}
                po_box = [None]
                for kb in range(kmax):
                    # S^T = k q^T, 2-head row-tiled pair, causally narrowed
                    cs = max(0, kb * 128 - qc * QC)
                    psS = ps_S.tile([128, 2, 512], F32, tag="S", bufs=2)
                    for hi in range(2):
                        nc.tensor.matmul(
                            psS[:, hi, cs:512],
                            lhsT=kT[hi * 64:(hi + 1) * 64, hp,
                                    kb * 128:(kb + 1) * 128],
                            rhs=qT[hi * 64:(hi + 1) * 64, hp,
                                   qc * QC + cs:(qc + 1) * QC],
                            start=True,
                            stop=True,
                        )
                    pT = work.tile([128, 2, 512], BF16, tag="pT", bufs=34)
                    pts[kb] = (pT, cs)
                    nc.scalar.activation(
                        pT[:, :, cs:512],
                        psS[:, :, cs:512],
                        mybir.ActivationFunctionType.Exp,
                        scale=SCALE,
                    )
                    if kb >= 4 * qc:  # diagonal band: zero the upper triangle
                        mask2 = bass.AP(
                            tensor=mask.tensor, offset=mask.offset,
                            ap=[mask.ap[0], [0, 2], mask.ap[1]],
                        )
                        nc.vector.tensor_mul(
                            pT[:, :, cs:cs + 128],
                            pT[:, :, cs:cs + 128],
                            mask2,
                        )
                    pump()
                # queue this phase's AV work + epilogue
                for kb in range(kmax):
                    workq.append((2, make_av(po_box, pts, kb, hp, kmax, qc)))
                workq.append((1, make_epilogue(po_box, oT, hp, qc=qc)))
                if qc in done_qc:
                    workq.extend((3, u) for u in make_out_proj(qc, oT))
                done_qc.add(qc)
                nqueued[0] = qpop[0] + len(workq)
                marks.append(nqueued[0])
        for _, u in workq:
            u()

    if _DBG:
        nc.sync.dma_start(out=_DBG["qdbg"], in_=qT[:, :, :])
        nc.sync.dma_start(out=_DBG["kdbg"], in_=kT[:, :, :])
        nc.sync.dma_start(out=_DBG["vdbg"], in_=v_sb[:, :, :, :])

    for p in [small, work, perm, const]:
        p.release()


_CACHE = {}
_DBG = {}


def _build():
    if "nc" in _CACHE:
        return _CACHE["nc"]
    nc = bacc.Bacc("TRN2", target_bir_lowering=False, debug=False, num_devices=8)
    import os
    if os.environ.get("KDBG"):
        _DBG["qdbg"] = nc.dram_tensor("qdbg", [128, 2, S], BF16, kind="ExternalOutput").ap()
        _DBG["kdbg"] = nc.dram_tensor("kdbg", [128, 2, S], BF16, kind="ExternalOutput").ap()
        _DBG["vdbg"] = nc.dram_tensor("vdbg", [128, NKB, NH, DK + 1], BF16, kind="ExternalOutput").ap()
        _DBG["edbg"] = nc.dram_tensor("edbg", [1, 3, S // 4], F32, kind="ExternalOutput").ap()
    xT_d = nc.dram_tensor("xT", [D, S], BF16, kind="ExternalInput").ap()
    wqT_d = nc.dram_tensor("wqT", [D, E], BF16, kind="ExternalInput").ap()
    wkT_d = nc.dram_tensor("wkT", [D, E], BF16, kind="ExternalInput").ap()
    wvT_d = nc.dram_tensor("wvT", [D, E], BF16, kind="ExternalInput").ap()
    woT_d = nc.dram_tensor("woT", [E, D], BF16, kind="ExternalInput").ap()
    yT_d = nc.dram_tensor("yT", [D, S], BF16, kind="ExternalOutput").ap()
    mask_d = nc.dram_tensor("maskc", [128, 128], BF16, kind="ExternalInput").ap()
    ones_d = nc.dram_tensor("onesc", [128, 64], BF16, kind="ExternalInput").ap()
    with tile.TileContext(nc) as tc:
        _emit(tc, nc, xT_d, wqT_d, wkT_d, wvT_d, woT_d, yT_d, mask_d, ones_d)
    nc.compile()
    _CACHE["nc"] = nc
    return nc


_r = np.arange(128)
_MASK = np.where(_r[:, None] <= _r[None, :], 1.0, 0.0).astype(ml_dtypes.bfloat16)
_ONES16 = np.ones((128, 64), dtype=ml_dtypes.bfloat16)

LAST_RESULT = None


def kernel(x, wq, wk, wv, wo):
    global LAST_RESULT
    nc = _build()
    bf = ml_dtypes.bfloat16
    x16 = np.asarray(x, dtype=np.float32).astype(bf)
    wq16 = np.asarray(wq, dtype=np.float32).astype(bf)
    wk16 = np.asarray(wk, dtype=np.float32).astype(bf)
    wv16 = np.asarray(wv, dtype=np.float32).astype(bf)
    wo16 = np.asarray(wo, dtype=np.float32).astype(bf)

    xTb = [np.ascontiguousarray(x16[b].T) for b in range(B)]
    in_maps = []
    for c in range(8):
        b, g = c // 4, c % 4
        rows = slice(g * E, (g + 1) * E)
        in_maps.append({
            "xT": xTb[b],
            "wqT": np.ascontiguousarray(wq16[rows].T),
            "wkT": np.ascontiguousarray(wk16[rows].T),
            "wvT": np.ascontiguousarray(wv16[rows].T),
            "woT": np.ascontiguousarray(wo16[:, rows].T),
            "maskc": _MASK,
            "onesc": _ONES16,
        })

    res = bass_utils.run_bass_kernel_spmd(nc, in_maps, core_ids=list(range(8)))
    LAST_RESULT = res

    y = np.empty((B, S, D), dtype=np.float32)
    for b in range(B):
        acc = res.results[4 * b]["yT"].astype(np.float32)
        for g in range(1, 4):
            acc += res.results[4 * b + g]["yT"].astype(np.float32)
        y[b] = acc.T
    return y


# revision 34
# speedup vs baseline: 1.0588x; 1.0025x over previous
"""Causal multi-head attention on 8 Trainium2 NeuronCores.

Sharding: data-parallel over batch (B=2) x tensor-parallel over heads
(16 heads -> 4 groups of 4). Core c handles batch c//4, head group c%4.
The host sums the 4 bf16 partial outputs per batch element in fp32.

All operand transposes (x^T, wq^T, wk^T, wv^T, wo^T) are done on the
HOST: the device never runs PE transposes, so phase 1 is pure
projection matmuls (dc-major, weights stationary across 4 s-chunks).

Matmuls run in bf16 (fp32 PSUM accumulation). The softmax row-sum is
fused into the o^T = [v|1s]^T P^T matmul via an appended ones column;
the reciprocal is taken on the single rowsum row and broadcast to 64
partitions with a K=1 bf16 matmul (fast: 2-byte moving operand).

Phase 2 processes q-chunks in order [0, 3, 2, 1]: the longest exp
chain (qc=3, 16 k-blocks) starts early while the shortest (qc=1)
forms the tail. Per (q-chunk, head-pair) only the S^T = k q^T matmuls
and the masked exp are emitted in the main loop; v-projection, AV
matmuls, normalization epilogues and output projections drain from a
FIFO work queue between them to keep the PE dense.
"""

import numpy as np
import ml_dtypes

import concourse.bacc as bacc
import concourse.bass as bass
import concourse.tile as tile
from concourse import bass_utils, mybir

B, S, D, H = 2, 2048, 1024, 16
DK = 64
NH = 4                 # heads per core
E = NH * DK            # 256: per-core head-dim slice
SCALE = 1.0 / 8.0      # 1/sqrt(DK)

F32 = mybir.dt.float32
BF16 = mybir.dt.bfloat16
FP8 = mybir.dt.float8e4
EXP_BIAS = -2.0        # exp(s/8 - 2): keeps P in fp8e4 range; cancels in
                       # the softmax normalization

QC = 512               # q-chunk (columns per attention tile)
NQC = S // QC          # 4
NKB = S // 128         # 16 k-blocks
QC_ORDER = [0, 3, 2, 1]


def _emit(tc, nc, xT_d, wqT_d, wkT_d, wvT_d, woT_d, yT_d, mask_d, ones_d):
    const = tc.alloc_tile_pool(name="const", bufs=1)
    perm = tc.alloc_tile_pool(name="perm", bufs=1)

    mask = const.tile([128, 128], BF16)
    nc.scalar.dma_start(out=mask, in_=mask_d)
    ones16 = const.tile([128, 64], BF16)
    nc.scalar.dma_start(out=ones16, in_=ones_d)

    # persistent sbuf tensors
    xT = perm.tile([128, 8, S], BF16)     # xT[p, dc, s] = x[s, dc*128+p]
    wqT = perm.tile([128, 8, E], BF16)    # wqT[p, dc, e] = wq[e, dc*128+p]
    wkT = perm.tile([128, 8, E], BF16)
    wvT = perm.tile([128, 8, E], BF16)
    woT = perm.tile([128, 2, D], BF16)    # woT[p, ec, o] = wo[o, ec*128+p]
    qT = perm.tile([128, 2, S], BF16)     # qT[p, hp, s]; p = hi*64+dk
    kT = perm.tile([128, 2, S], BF16)
    v_sb = perm.tile([128, NKB, NH, 128], BF16)  # [.., 64] = ones column

    # interleave weight + x chunk loads so the first projection pass can
    # start after the first (wq, wk, x-quarter) triple lands; x chunks are
    # split into s-quarters to match the dc-major consumption order
    for dc in range(8):
        nc.sync.dma_start(out=wqT[:, dc, :], in_=wqT_d[dc * 128:(dc + 1) * 128, :])
        nc.sync.dma_start(out=wkT[:, dc, :], in_=wkT_d[dc * 128:(dc + 1) * 128, :])
        nc.sync.dma_start(out=xT[:, dc, :], in_=xT_d[dc * 128:(dc + 1) * 128, :])
    for dc in range(8):
        nc.sync.dma_start(out=wvT[:, dc, :], in_=wvT_d[dc * 128:(dc + 1) * 128, :])
    for ec in range(2):
        nc.sync.dma_start(out=woT[:, ec, :], in_=woT_d[ec * 128:(ec + 1) * 128, :])

    # ones column of v (written once; strided 3D AP)
    ones_ap = bass.AP(
        tensor=v_sb.tensor,
        offset=v_sb.offset + DK,
        ap=[v_sb.ap[0], [NH * 128, NKB], [128, NH]],
    )
    nc.vector.memset(ones_ap, 1.0)

    ncopy = [0]

    def copy(dst, src):
        # alternate psum->sbuf copies across DVE and ACT in phase 1
        if ncopy[0] % 2 == 0:
            nc.vector.tensor_copy(dst, src)
        else:
            nc.scalar.copy(dst, src)
        ncopy[0] += 1

    work = tc.alloc_tile_pool(name="work", bufs=3)
    small = tc.alloc_tile_pool(name="small", bufs=2)

    # ---- phase 1: q/k projections, dc-major (weights stationary
    # across the 4 s-chunks -> hardware may skip weight reloads) ----
    with tc.tile_pool(name="psP", bufs=1, space="PSUM") as psP:
        def proj_pass(w_t, outT, ec):
            ps = [psP.tile([128, QC], F32, tag="proj", bufs=8, name="psp")
                  for _ in range(4)]
            for dc in range(8):
                for sc in range(4):
                    nc.tensor.matmul(
                        ps[sc],
                        lhsT=w_t[:, dc, ec * 128:(ec + 1) * 128],
                        rhs=xT[:, dc, sc * QC:(sc + 1) * QC],
                        start=(dc == 0),
                        stop=(dc == 7),
                    )
            for sc in range(4):
                copy(outT[:, ec, sc * QC:(sc + 1) * QC], ps[sc])

        proj_pass(wqT, qT, 0)
        proj_pass(wkT, kT, 0)

    with tc.tile_pool(name="psS", bufs=1, space="PSUM") as ps_S, \
         tc.tile_pool(name="psO", bufs=1, space="PSUM") as ps_o, \
         tc.tile_pool(name="psY", bufs=1, space="PSUM") as ps_y:

        # ---- phase 2: attention + output projection, FIFO work queue ----
        workq = []  # (weight, closure); FIFO order guarantees v before AV

        def make_proj_unit(w_t, outT, ec, sc):
            def u():
                ps = ps_y.tile([128, QC], F32, tag="y", bufs=2, name="psu")
                for dc in range(8):
                    nc.tensor.matmul(
                        ps,
                        lhsT=w_t[:, dc, ec * 128:(ec + 1) * 128],
                        rhs=xT[:, dc, sc * QC:(sc + 1) * QC],
                        start=(dc == 0),
                        stop=(dc == 7),
                    )
                nc.vector.tensor_copy(outT[:, ec, sc * QC:(sc + 1) * QC], ps)
            return u

        def make_vproj(sblk):
            def u():
                ps = ps_y.tile([128, E], F32, tag="y", bufs=2, name="psv")
                for dc in range(8):
                    nc.tensor.matmul(
                        ps,
                        lhsT=xT[:, dc, sblk * 128:(sblk + 1) * 128],
                        rhs=wvT[:, dc, :],
                        start=(dc == 0),
                        stop=(dc == 7),
                    )
                # scatter 4 heads into [.., l, 0:64] (DVE: strided psum read)
                sap = bass.AP(
                    tensor=ps.tensor, offset=ps.offset,
                    ap=[ps.ap[0], [DK, NH], [1, DK]],
                )
                nc.vector.tensor_copy(v_sb[:, sblk, :, 0:DK], sap)
            return u

        # interleave vproj with ec1 proj units so the heavyweight proj
        # drains spread across the early slots (v0-3 first: qc0 AVs)
        punits = [make_proj_unit(w_t, outT, 1, sc)
                  for w_t, outT in ((wqT, qT), (wkT, kT)) for sc in range(4)]
        vunits = [make_vproj(sblk) for sblk in range(NKB)]
        order = vunits[0:4] + [punits[0]]
        vi, pi = 4, 1
        while vi < NKB or pi < 8:
            order.extend(vunits[vi:vi + 2])
            vi += 2
            if pi < 8:
                order.append(punits[pi])
                pi += 1
        for u in order:
            workq.append((6 if u in vunits else 10, u))

        # drain pacing: spread background work evenly over the QK slots
        TOTAL_SLOTS = 2 * sum(4 * (q + 1) for q in range(NQC))      # 80
        W_TOTAL = 6 * NKB + 10 * 8 + 2 * TOTAL_SLOTS + 2 * NQC + 3 * 8 * NQC
        slot_i = [0]
        drained = [0]
        qpop = [0]          # units popped (for watermark guard)
        nqueued = [len(workq)]

        def drain_one():
            w, u = workq.pop(0)
            u()
            drained[0] += w
            qpop[0] += 1
            return w

        N_PRE_W = 6 * NKB + 10 * 8  # early work: vproj + ec1 proj units

        def pump():
            slot_i[0] += 1
            target = (W_TOTAL * slot_i[0]) // TOTAL_SLOTS
            # drain the pre-work fast enough that the (0,h1) boundary
            # never bulk-drains with ACT idle
            target = max(target, min(N_PRE_W, 9 * slot_i[0]))
            budget = min(14, target - drained[0])
            while workq and budget > 0:
                budget -= drain_one()

        def make_av(po_box, pts, kb, hp, kmax, qc):
            def av():
                if po_box[0] is None:
                    po_box[0] = (
                        ps_o.tile([128, QC], F32, tag="o", bufs=2, name="poA"),
                        ps_o.tile([128, QC], F32, tag="o", bufs=2, name="poB"),
                    )
                poA, poB = po_box[0]
                pT, cs = pts[kb]
                for hi, po in ((0, poA), (1, poB)):
                    # 128-col weights (v padded with 64 dead columns) keep
                    # FWL on; po rows 65:127 hold garbage and are never read
                    nc.tensor.matmul(
                        po[:, cs:512],
                        lhsT=v_sb[:, kb, 2 * hp + hi, :],
                        rhs=pT[:, hi, cs:512],
                        start=(kb == 0),
                        stop=(kb == kmax - 1),
                    )
            return av

        def make_epilogue(po_box, oT, hp, qc=None):
            def epi():
                poA, poB = po_box[0]
                # free the psum banks quickly with one copy per head
                oA_sb = small.tile([DK + 1, QC], F32, tag="osb", bufs=4)
                oB_sb = small.tile([DK + 1, QC], F32, tag="osb", bufs=4)
                if qc == 1:
                    # tail chain: ACT's exp queue is empty by now
                    nc.scalar.copy(oA_sb, poA[0:DK + 1, :])
                    nc.scalar.copy(oB_sb, poB[0:DK + 1, :])
                else:
                    nc.vector.tensor_copy(oA_sb, poA[0:DK + 1, :])
                    nc.vector.tensor_copy(oB_sb, poB[0:DK + 1, :])
                for hi, o_sb in ((0, oA_sb), (1, oB_sb)):
                    # broadcast the rowsum row (partition 64) to 64
                    # partitions via K=1 bf16 matmul (2-byte moving operand
                    # streams 2x faster than fp32), then reciprocal
                    sum16 = small.tile([DK + 1, QC], BF16, tag="rec", bufs=2)
                    nc.vector.tensor_copy(sum16[DK:DK + 1, :], o_sb[DK:DK + 1, :])
                    ps_bc = ps_y.tile([64, QC], F32, tag="y", bufs=2, name="psbc")
                    nc.tensor.matmul(
                        ps_bc,
                        lhsT=ones16[64:65, :],
                        rhs=sum16[DK:DK + 1, :],
                        start=True,
                        stop=True,
                    )
                    rec = small.tile([64, QC], F32, tag="recf", bufs=2)
                    nc.vector.reciprocal_approx_fast(rec, ps_bc)
                    if hi == 0:
                        nc.vector.tensor_mul(oT[0:64, hp, :], o_sb[0:DK, :], rec)
                    else:
                        tmpB = small.tile([64, QC], BF16, tag="tmpB", bufs=2)
                        nc.vector.tensor_mul(tmpB, o_sb[0:DK, :], rec)
                        # partition shift 0-63 -> 64-127 via sbuf->sbuf DMA
                        nc.sync.dma_start(out=oT[64:128, hp, :], in_=tmpB)
            return epi

        def make_out_proj(qc, oT):
            units = []
            for dc in range(8):
                def u(dc=dc, qc=qc, oT=oT):
                    psy = ps_y.tile([128, QC], F32, tag="y", bufs=2, name="psy")
                    for ec in range(2):
                        nc.tensor.matmul(
                            psy,
                            lhsT=woT[:, ec, dc * 128:(dc + 1) * 128],
                            rhs=oT[:, ec, :],
                            start=(ec == 0),
                            stop=(ec == 1),
                        )
                    y_sb = work.tile([128, QC], BF16, tag="ysb", bufs=3)
                    if qc == 1:
                        nc.scalar.copy(y_sb, psy)
                    else:
                        nc.vector.tensor_copy(y_sb, psy)
                    nc.sync.dma_start(
                        out=yT_d[dc * 128:(dc + 1) * 128, qc * QC:(qc + 1) * QC],
                        in_=y_sb,
                    )
                units.append(u)
            return units

        marks = []  # cumulative queued-unit count at each chain's end
        CHAINS = [(0, 0), (3, 0), (0, 1), (3, 1), (2, 0), (2, 1), (1, 0), (1, 1)]
        N_PRE = len(workq)  # vproj + proj-ec1 units
        oTs = {}
        done_qc = set()
        for ci, (qc, hp) in enumerate(CHAINS):
            if qc not in oTs:
                oTs[qc] = work.tile([128, 2, QC], BF16, tag="oT", bufs=3, name="oT")
            oT = oTs[qc]
            kmax = 4 * (qc + 1)
            if True:
                # ec1 q/k projections must be emitted before any hp=1 chain
                if hp == 1:
                    while qpop[0] < N_PRE:
                        drain_one()
                # guard: chains up to N-2 fully drained before N emits, so
                # at most two chains' pT tiles (<=32) are live in the ring
                if len(marks) >= 2:
                    while qpop[0] < marks[-2]:
                        drain_one()
                pts = {}
                po_box = [None]
                for kb in range(kmax):
                    # S^T = k q^T, 2-head row-tiled pair, causally narrowed
                    cs = max(0, kb * 128 - qc * QC)
                    psS = ps_S.tile([128, 2, 512], F32, tag="S", bufs=2)
                    for hi in range(2):
                        nc.tensor.matmul(
                            psS[:, hi, cs:512],
                            lhsT=kT[hi * 64:(hi + 1) * 64, hp,
                                    kb * 128:(kb + 1) * 128],
                            rhs=qT[hi * 64:(hi + 1) * 64, hp,
                                   qc * QC + cs:(qc + 1) * QC],
                            start=True,
                            stop=True,
                        )
                    pT = work.tile([128, 2, 512], BF16, tag="pT", bufs=34)
                    pts[kb] = (pT, cs)
                    nc.scalar.activation(
                        pT[:, :, cs:512],
                        psS[:, :, cs:512],
                        mybir.ActivationFunctionType.Exp,
                        scale=SCALE,
                    )
                    if kb >= 4 * qc:  # diagonal band: zero the upper triangle
                        mask2 = bass.AP(
                            tensor=mask.tensor, offset=mask.offset,
                            ap=[mask.ap[0], [0, 2], mask.ap[1]],
                        )
                        nc.vector.tensor_mul(
                            pT[:, :, cs:cs + 128],
                            pT[:, :, cs:cs + 128],
                            mask2,
                        )
                    pump()
                # queue this phase's AV work + epilogue
                for kb in range(kmax):
                    workq.append((2, make_av(po_box, pts, kb, hp, kmax, qc)))
                workq.append((1, make_epilogue(po_box, oT, hp, qc=qc)))
                if qc in done_qc:
                    workq.extend((3, u) for u in make_out_proj(qc, oT))
                done_qc.add(qc)
                nqueued[0] = qpop[0] + len(workq)
                marks.append(nqueued[0])
        for _, u in workq:
            u()

    if _DBG:
        nc.sync.dma_start(out=_DBG["qdbg"], in_=qT[:, :, :])
        nc.sync.dma_start(out=_DBG["kdbg"], in_=kT[:, :, :])
        nc.sync.dma_start(out=_DBG["vdbg"], in_=v_sb[:, :, :, :])

    for p in [small, work, perm, const]:
        p.release()


_CACHE = {}
_DBG = {}


def _build():
    if "nc" in _CACHE:
        return _CACHE["nc"]
    nc = bacc.Bacc("TRN2", target_bir_lowering=False, debug=False, num_devices=8)
    import os
    if os.environ.get("KDBG"):
        _DBG["qdbg"] = nc.dram_tensor("qdbg", [128, 2, S], BF16, kind="ExternalOutput").ap()
        _DBG["kdbg"] = nc.dram_tensor("kdbg", [128, 2, S], BF16, kind="ExternalOutput").ap()
        _DBG["vdbg"] = nc.dram_tensor("vdbg", [128, NKB, NH, DK + 1], BF16, kind="ExternalOutput").ap()
        _DBG["edbg"] = nc.dram_tensor("edbg", [1, 3, S // 4], F32, kind="ExternalOutput").ap()
    xT_d = nc.dram_tensor("xT", [D, S], BF16, kind="ExternalInput").ap()
    wqT_d = nc.dram_tensor("wqT", [D, E], BF16, kind="ExternalInput").ap()
    wkT_d = nc.dram_tensor("wkT", [D, E], BF16, kind="ExternalInput").ap()
    wvT_d = nc.dram_tensor("wvT", [D, E], BF16, kind="ExternalInput").ap()
    woT_d = nc.dram_tensor("woT", [E, D], BF16, kind="ExternalInput").ap()
    yT_d = nc.dram_tensor("yT", [D, S], BF16, kind="ExternalOutput").ap()
    mask_d = nc.dram_tensor("maskc", [128, 128], BF16, kind="ExternalInput").ap()
    ones_d = nc.dram_tensor("onesc", [128, 64], BF16, kind="ExternalInput").ap()
    with tile.TileContext(nc) as tc:
        _emit(tc, nc, xT_d, wqT_d, wkT_d, wvT_d, woT_d, yT_d, mask_d, ones_d)
    nc.compile()
    _CACHE["nc"] = nc
    return nc


_r = np.arange(128)
_MASK = np.where(_r[:, None] <= _r[None, :], 1.0, 0.0).astype(ml_dtypes.bfloat16)
_ONES16 = np.ones((128, 64), dtype=ml_dtypes.bfloat16)

LAST_RESULT = None


def kernel(x, wq, wk, wv, wo):
    global LAST_RESULT
    nc = _build()
    bf = ml_dtypes.bfloat16
    x16 = np.asarray(x, dtype=np.float32).astype(bf)
    wq16 = np.asarray(wq, dtype=np.float32).astype(bf)
    wk16 = np.asarray(wk, dtype=np.float32).astype(bf)
    wv16 = np.asarray(wv, dtype=np.float32).astype(bf)
    wo16 = np.asarray(wo, dtype=np.float32).astype(bf)

    xTb = [np.ascontiguousarray(x16[b].T) for b in range(B)]
    in_maps = []
    for c in range(8):
        b, g = c // 4, c % 4
        rows = slice(g * E, (g + 1) * E)
        in_maps.append({
            "xT": xTb[b],
            "wqT": np.ascontiguousarray(wq16[rows].T),
            "wkT": np.ascontiguousarray(wk16[rows].T),
            "wvT": np.ascontiguousarray(wv16[rows].T),
            "woT": np.ascontiguousarray(wo16[:, rows].T),
            "maskc": _MASK,
            "onesc": _ONES16,
        })

    res = bass_utils.run_bass_kernel_spmd(nc, in_maps, core_ids=list(range(8)))
    LAST_RESULT = res

    y = np.empty((B, S, D), dtype=np.float32)
    for b in range(B):
        acc = res.results[4 * b]["yT"].astype(np.float32)
        for g in range(1, 4):
            acc += res.results[4 * b + g]["yT"].astype(np.float32)
        y[b] = acc.T
    return y
